# revision 32
# baseline (speedup 1.0000x reference)
"""Depth-masked 3-branch 3x3 conv (Conv2.5D) on 8 TRN2 NeuronCores.

Data-parallel over N=8 images (1 image/core). Per core, v2 design:
  - x is host-prepped into two bf16 slabs tA=[x; x+1], tB=[x; x+128]
    (zero-padded), DMA'd straight to SBUF: no on-chip staging/conversion
  - phi in {0,1,2,3} encodes the active branch per (tap,pixel); computed
    once in f32 on DVE, collapsed to 8 pixel-major rc rows (one per tap)
  - per-chunk phi broadcast (row -> 64 partitions, both taps of a pair)
    is a single SBUF->SBUF DMA with a 0-stride middle dim: no PE/Act work
  - pair bases: p0,p1 E-basis {phi*x, E2*x, E3*x} (E_j built by DVE
    tensor_scalar at 4x rate); p2,p3 sign-basis {phi*x, s2*x, s3*x, x}
    (signs from Act, raw-x group free) -> 15 GEMM groups, 60 matmuls
    per 2048-px superchunk
  - multiply split: phi*x of p1,p2,p3 on Pool; the other 9 on DVE
  - out accumulated in PSUM, Act-copied to bf16, host casts to f32
"""

import sys

sys.path.insert(0, "/opt/trn_rl_repo")

import numpy as np
import ml_dtypes

import concourse.bass as bass
import concourse.mybir as mybir
from concourse.bass_utils import run_bass_kernel_spmd
from concourse import tile
from concourse.vector_clock import VectorClock, ScopedClock

F32 = mybir.dt.float32
BF16 = mybir.dt.bfloat16
AF = mybir.ActivationFunctionType
ALU = mybir.AluOpType

N_IMG, C, O, H, W = 8, 64, 64, 128, 128
L = H * W
CHUNK = 2048
NCHUNK = L // CHUNK
BASE = 144  # pad on each side of the x slabs (window reads span +-129)
XW = BASE + L + BASE
# tap k = 3*(dh+1)+(dw+1); flat pixel offset dh*W+dw
OFF = [(k // 3 - 1) * W + (k % 3 - 1) for k in range(9)]
# tap pairs (ka,kb): off(kb)-off(ka) == 1 -> tA, == 128 -> tB
PAIRS = [(0, 1), (7, 8), (2, 5), (3, 6)]
PAIR_SRC = ["A", "A", "B", "B"]
NGRP = 13  # center + 3 per pair


def _patched_drain_and_barrier(self, tick_clock, wait_clock):
    # stock version puts every live sem wait on one drain -> walrus
    # "Too many sync wait commands"; emit one single-wait NOP per sem.
    ticks = list(tick_clock.global_clock)
    n = len(ticks)
    for i, t in enumerate(ticks):
        if t > 0:
            vec = [0] * n
            vec[i] = t
            nop = self.nc.sync.nop()
            wait_clock.add_sem_waits(nop.ins, ScopedClock({None: VectorClock(vec)}))
    self.nc.sync.drain()
    self.nc.all_engine_barrier()
    popped = self.nc._tile_sem_poison_stack.pop()
    assert popped is self._sem_poison
    self.nc.clear_and_free_semaphores(list(self.sems.allocated().values()))
    self.nc.all_engine_barrier()


tile.TileContext._drain_and_barrier = _patched_drain_and_barrier


def _split_excess_waits(nc, noop_cls, max_waits=1):
    # this walrus build rejects >1 sync-wait on several instruction
    # structs; hoist extras onto same-engine NoOps placed just before.
    for fn in nc.m.functions:
        for blk in fn.blocks:
            idx = 0
            while idx < len(blk.instructions):
                inst = blk.instructions[idx]
                si = inst.sync_info
                if si is not None and len(si.on_wait) > max_waits:
                    waits = list(si.on_wait)
                    si.on_wait = waits[-max_waits:]
                    pos = idx
                    for w in waits[:-max_waits]:
                        nop = noop_cls(
                            name=nc.get_next_instruction_name(), ins=[], outs=[]
                        )
                        nop.engine = inst.engine
                        nop.sync_info = mybir.SyncInfo(on_wait=[w], on_update=[])
                        nc.register_instruction(nop)
                        blk.instructions.insert(pos, nop)
                        pos += 1
                        idx += 1
                idx += 1


def _build_graph():
    nc = bass.Bass()
    xt_d = nc.declare_dram_parameter("xt", [128, 2 * XW], BF16, isOutput=False)
    # host-packed: cols 0:390 = dsh (3 shifted depth copies, zero-edged),
    # col 390 = 1/fx per partition
    dsh_d = nc.declare_dram_parameter("dshp", [128, 391], F32, isOutput=False)
    wp_d = nc.declare_dram_parameter("wp", [128, NGRP * 64], BF16, isOutput=False)
    out_d = nc.declare_dram_parameter("out", [O, L], BF16, isOutput=True)

    with tile.TileContext(nc) as tc:
        WIN = CHUNK + 264  # slab window: [BASE+c0-132, BASE+c0+CHUNK+132)
        with (
            tc.tile_pool(name="big", bufs=1) as big,
            tc.tile_pool(name="slab", bufs=3) as slb,
            tc.tile_pool(name="mask", bufs=1) as mk,
            tc.tile_pool(name="scr", bufs=3) as scr,
            tc.tile_pool(name="rrep", bufs=10) as rrp,
            tc.tile_pool(name="xm", bufs=8) as xmp,
            tc.tile_pool(name="sgn", bufs=6) as sgp,
            tc.tile_pool(name="ebp", bufs=3) as ebp,
            tc.tile_pool(name="outp", bufs=2) as outp,
            tc.tile_pool(name="psum", bufs=2, space=bass.MemorySpace.PSUM) as psp,
        ):
            # ---- x slabs: host-prepped bf16, zero-padded; tA rows 0:64 =
            # x, 64:128 = x shifted +1; tB rows 64:128 = x shifted +128.
            # Per-superchunk windows in a ring: tile cols [0,WIN) = tA
            # window, [WIN,2*WIN) = tB window; col 132 = global BASE+c0 ----
            def load_slabs(ci):
                g0 = BASE + ci * CHUNK - 132
                xw = slb.tile([128, 2 * WIN], BF16, tag="xw")
                src = bass.AP(
                    xt_d[:].tensor, xt_d[:].offset + g0,
                    [list(xt_d[:].ap[0]), [XW, 2], [1, WIN]],
                )
                dst = bass.AP(
                    xw[:].tensor, xw[:].offset,
                    [list(xw[:].ap[0]), [WIN, 2], [1, WIN]],
                )
                nc.sync.dma_start(dst, src)
                return xw

            # depth DMA first: it heads the critical path (phi encode)
            dshp = mk.tile([128, 391], F32)
            nc.sync.dma_start(dshp[:], dsh_d[:])
            rfx = dshp[:, 390:391]
            wp = big.tile([128, NGRP * 64], BF16)
            nc.scalar.dma_start(wp[:], wp_d[:])
            slab_tiles = {0: load_slabs(0), 1: load_slabs(1)}
            # per-partition bias vectors for the AF.Relu basis planes:
            # relu(2*phi-3) = {0,0,1,3}, relu(2*phi-5) = {0,0,0,1}
            b3 = mk.tile([128, 1], F32)
            nc.vector.memset(b3[:], -3.0)
            b5 = mk.tile([128, 1], F32)
            nc.vector.memset(b5[:], -5.0)

            # ---- depth -> phi encoding, split into two 4-tap halves so
            # pair p0/p1 broadcasts start while p2/p3 still encode.
            # Half A = taps (0,1,7,8) = pairs p0,p1: dsh offsets
            # {0,1,261,262}; half B = taps (2,5,3,6) = pairs p2,p3:
            # offsets {2,132,130,260} ----
            g = mk.tile([128, 128], F32)
            nc.vector.tensor_scalar(g[:], dshp[:, 131:259], rfx, None, ALU.mult)
            rg = mk.tile([128, 128], F32)
            nc.vector.reciprocal(rg[:], g[:])

            def _win(base, offset, dims):
                return bass.AP(
                    base.tensor, offset, [list(base.ap[0])] + [list(d) for d in dims]
                )

            rc = big.tile([8, L], BF16)
            rgb4 = _win(rg[:], 0, [(0, 4), (1, 128)])
            cent4 = _win(dshp[:], 131, [(0, 4), (1, 128)])

            def encode_half(dims, off):
                dcol = _win(dshp[:], off, dims + [(1, 128)])
                et = scr.tile([128, 512], F32, tag="u")
                nc.vector.tensor_tensor(et[:], dcol, cent4, ALU.subtract)
                tq = scr.tile([128, 512], F32, tag="t")
                nc.vector.tensor_tensor(tq[:], et[:], rgb4, ALU.mult)
                ua = scr.tile([128, 512], F32, tag="u")
                nc.vector.tensor_scalar(ua[:], tq[:], -1.5, None, ALU.is_ge)
                ub = scr.tile([128, 512], F32, tag="u")
                nc.vector.scalar_tensor_tensor(
                    ub[:], tq[:], -0.5, ua[:], ALU.is_ge, ALU.add
                )
                uc = scr.tile([128, 512], F32, tag="u")
                nc.vector.scalar_tensor_tensor(
                    uc[:], tq[:], 0.5, ub[:], ALU.is_ge, ALU.add
                )
                renc = scr.tile([128, 512], BF16, tag="r")
                nc.vector.scalar_tensor_tensor(
                    renc[:], tq[:], 1.5, uc[:], ALU.is_lt, ALU.mult
                )
                return renc

            def collapse_pair(renc, i0, r0):
                for i in (i0, i0 + 1):
                    r = r0 + (i - i0)
                    eng = nc.sync if i % 2 == 0 else nc.scalar
                    eng.dma_start(
                        rc[r : r + 1, :], renc[:, i * 128 : (i + 1) * 128]
                    )

            # phi broadcast for superchunk ci, pairs [plo,phi): one DMA per
            # pair; row 2p -> partitions 0:64, row 2p+1 -> 64:128
            def bcast(ci, plo=0, phi=4, rrs=None):
                c0 = ci * CHUNK
                rrs = rrs if rrs is not None else [None] * 4
                for p in range(plo, phi):
                    rr = rrp.tile([128, CHUNK], BF16, tag="rr")
                    b = rc[2 * p : 2 * p + 2, c0 : c0 + CHUNK]
                    src = bass.AP(
                        b.tensor, b.offset,
                        [list(b.ap[0]), [0, 64], [1, CHUNK]],
                    )
                    eng = nc.sync if p % 2 == 0 else nc.scalar
                    eng.dma_start(rr[:], src)
                    rrs[p] = rr
                return rrs

            ra = encode_half([(261, 2), (1, 2)], 0)
            collapse_pair(ra, 0, 0)
            pipe = bcast(0, 0, 1)
            collapse_pair(ra, 2, 2)
            bcast(0, 1, 2, pipe)
            rb = encode_half([(128, 2), (130, 2)], 2)
            collapse_pair(rb, 0, 4)
            bcast(0, 2, 3, pipe)
            collapse_pair(rb, 2, 6)
            bcast(0, 3, 4, pipe)

            # logical groups: 0=center; pair p slots 1+3p..3+3p = {f1,f2,f3}
            # f1 = phi*x; p0,p1: f2,f3 = E2*x, E3*x (DVE tensor_scalar);
            # p2,p3: f2,f3 = relu(2phi-3)*x, relu(2phi-5)*x (Act planes)
            pending = None
            for ci in range(NCHUNK):
                c0 = ci * CHUNK
                rr = pipe
                xw = slab_tiles.pop(ci)

                def xwin_of(p):
                    off = (0 if PAIR_SRC[p] == "A" else WIN) + 132
                    off += OFF[PAIRS[p][0]]
                    return xw[:, off : off + CHUNK]

                xms = {}

                # ---- Pool: phi*x for p1, p2, p3 (slow; consumed last) ----
                for p in (1, 2, 3):
                    xm = xmp.tile([128, CHUNK], BF16, tag="xm")
                    nc.gpsimd.tensor_tensor(xm[:], rr[p][:], xwin_of(p), ALU.mult)
                    xms[1 + 3 * p] = xm

                # ---- Act: relu basis planes: (p,j) in (1,2),(2,2),(2,3),
                # (3,2),(3,3); p1 f3 stays an E3 plane on DVE ----
                sg = {}
                for p, j, bv in (
                    (1, 2, b3), (2, 2, b3), (2, 3, b5), (3, 2, b3), (3, 3, b5)
                ):
                    s = sgp.tile([128, CHUNK], BF16, tag="sg")
                    nc.scalar.activation(
                        s[:], rr[p][:], AF.Relu, bias=bv[:], scale=2.0
                    )
                    sg[(p, j)] = s

                # previous super's PSUM -> SBUF copy + store, emitted after
                # this super's Act planes so it never delays them
                if pending is not None:
                    pacc, pc0 = pending
                    osb = outp.tile([O, CHUNK], BF16, tag="osb")
                    nc.scalar.activation(osb[:], pacc[:], AF.Copy)
                    nc.sync.dma_start(out_d[:, pc0 : pc0 + CHUNK], osb[:])

                # ---- DVE: E planes + 9 multiplies ----
                def dve_tt(gid, a, p):
                    xm = xmp.tile([128, CHUNK], BF16, tag="xm")
                    nc.vector.tensor_tensor(xm[:], a[:], xwin_of(p), ALU.mult)
                    xms[gid] = xm

                # E planes first: DVE then streams multiplies gap-free at
                # a rate above PE consumption, so PE keeps a backlog
                ebs = {}
                for p, j in ((0, 2), (0, 3), (1, 3)):
                    eb = ebp.tile([128, CHUNK], BF16, tag="eb")
                    nc.vector.tensor_scalar(
                        eb[:], rr[p][:], j - 0.5, None, ALU.is_ge
                    )
                    ebs[(p, j)] = eb
                dve_tt(1, rr[0], 0)  # p0 f1 = phi*x
                dve_tt(2, ebs[(0, 2)], 0)
                dve_tt(3, ebs[(0, 3)], 0)
                dve_tt(6, ebs[(1, 3)], 1)
                dve_tt(5, sg[(1, 2)], 1)
                dve_tt(8, sg[(2, 2)], 2)
                if ci + 1 < NCHUNK:
                    pipe = bcast(ci + 1)
                if ci + 2 < NCHUNK:
                    slab_tiles[ci + 2] = load_slabs(ci + 2)
                dve_tt(9, sg[(2, 3)], 2)
                dve_tt(11, sg[(3, 2)], 3)
                dve_tt(12, sg[(3, 3)], 3)

                # ---- matmuls: 13 groups x 4 PSUM banks ----
                # order: center first (free rhs), DVE tiles in build order,
                # Pool tiles last
                acc = psp.tile([O, CHUNK], F32)
                mm_rhs = [(0, xw[:, 132 : 132 + CHUNK])]
                for gid in (1, 2, 4, 3, 6, 5, 7, 8, 9, 11, 12, 10):
                    mm_rhs.append((gid, xms[gid][:]))

                nmm = len(mm_rhs)
                MMN = 512
                for oi, (gid, rhs) in enumerate(mm_rhs):
                    for h in range(CHUNK // MMN):
                        nc.tensor.matmul(
                            acc[:, h * MMN : (h + 1) * MMN],
                            wp[:, gid * 64 : (gid + 1) * 64],
                            bass.AP(
                                rhs.tensor,
                                rhs.offset + h * MMN,
                                [list(rhs.ap[0])] + [[1, MMN]],
                            ),
                            start=(oi == 0),
                            stop=(oi == nmm - 1),
                        )
                pending = (acc, c0)

            pacc, pc0 = pending
            osb = outp.tile([O, CHUNK], BF16, tag="osb")
            nc.scalar.activation(osb[:], pacc[:], AF.Copy)
            nc.sync.dma_start(out_d[:, pc0 : pc0 + CHUNK], osb[:])

    noop_cls = type(nc.sync.nop().ins)
    _split_excess_waits(nc, noop_cls, max_waits=1)
    return nc


def _bf(a):
    return a.astype(ml_dtypes.bfloat16).astype(np.float32)


def _prep_weights(w0, w1, w2):
    # basis-transformed weights; see module docstring.
    # E-basis (p0,p1):  V1=W2, V2=W1-2W2, V3=W0-W1-W2 for {phi,E2,E3}
    # sign (p2,p3) {f1,s2,s3,xr}: U1=W2, U2=.5W1-W2, U3=.5(W0-W1-W2),
    #   U4=U2+U3 (from the bf16-rounded terms so phi=0 pixels cancel
    #   exactly in the f32 PSUM accumulation)
    ws = [w0.reshape(O, C, 9), w1.reshape(O, C, 9), w2.reshape(O, C, 9)]
    W0, W1, W2 = ws
    wp = np.zeros((128, NGRP * 64), dtype=np.float32)

    def put(gi, ka, kb, arr):
        wp[0:64, gi * 64 : (gi + 1) * 64] = arr[:, :, ka].T
        if kb is not None:
            wp[64:128, gi * 64 : (gi + 1) * 64] = arr[:, :, kb].T

    put(0, 4, None, W1)  # center
    # E-basis (p0,p1) over {phi, E2, E3}; relu-basis (p2,p3) over
    # {phi, relu(2phi-3), relu(2phi-5)} = {(1,2,3),(0,1,3),(0,0,1)} at
    # phi=1,2,3 -> R = [W2, W1-2W2, W0-3W1+3W2]
    V = [W2, W1 - 2 * W2, W0 - W1 - W2]
    R = [W2, W1 - 2 * W2, W0 - 3 * W1 + 3 * W2]
    for p in range(4):
        ka, kb = PAIRS[p]
        basis = V if p == 0 else R
        for j in range(3):
            put(1 + p * 3 + j, ka, kb, basis[j])
    return wp.astype(ml_dtypes.bfloat16)


def _prep_dsh(dep, fxi):
    # [128, 391] f32: 3 x 130-col blocks of row-shifted depth (dh=-1,0,+1,
    # zero edges, 1-col left pad within each block) + col 390 = 1/fx
    d = np.zeros((128, 391), dtype=np.float32)
    d[1:128, 1:129] = dep[0:127]
    d[:, 131:259] = dep
    d[0:127, 261:389] = dep[1:128]
    d[:, 390] = 1.0 / fxi
    return d


def _prep_x(x):
    # x: [C, L] f32 -> [128, 2*XW] bf16: tA = [x; x+1], tB = [x; x+128]
    xp = np.zeros((C, XW + 130), dtype=np.float32)
    xp[:, BASE : BASE + L] = x
    xt = np.zeros((128, 2 * XW), dtype=np.float32)
    xt[0:64, 0:XW] = xp[:, 0:XW]
    xt[64:128, 0:XW] = xp[:, 1 : XW + 1]
    xt[0:64, XW:] = xp[:, 0:XW]
    xt[64:128, XW:] = xp[:, 128 : XW + 128]
    return xt.astype(ml_dtypes.bfloat16)


def kernel(x, depth, fx, weight_0, weight_1, weight_2, _trace=False):
    x = np.asarray(x, dtype=np.float32)
    depth = np.asarray(depth, dtype=np.float32)
    fx = np.asarray(fx, dtype=np.float32)
    wp = _prep_weights(
        np.asarray(weight_0, np.float32),
        np.asarray(weight_1, np.float32),
        np.asarray(weight_2, np.float32),
    )
    in_maps = []
    for i in range(N_IMG):
        in_maps.append(
            {
                "xt": _prep_x(x[i].reshape(C, L)),
                "dshp": _prep_dsh(depth[i, 0], fx[i]),
                "wp": wp,
            }
        )
    nc = _build_graph()
    res = run_bass_kernel_spmd(nc, in_maps, core_ids=list(range(N_IMG)), trace=_trace)
    out = np.stack(
        [
            res.results[i]["out"].astype(np.float32).reshape(O, H, W)
            for i in range(N_IMG)
        ]
    )
    if _trace:
        return out, res
    return out


if __name__ == "__main__":
    rng = np.random.default_rng(0)
    ins = {
        "x": rng.standard_normal((N_IMG, C, H, W), dtype=np.float32),
        "depth": (1.0 + 9.0 * rng.random((N_IMG, 1, H, W))).astype(np.float32),
        "fx": (400.0 + 200.0 * rng.random(N_IMG)).astype(np.float32),
        "weight_0": rng.standard_normal((O, C, 3, 3), dtype=np.float32) * 0.04,
        "weight_1": rng.standard_normal((O, C, 3, 3), dtype=np.float32) * 0.04,
        "weight_2": rng.standard_normal((O, C, 3, 3), dtype=np.float32) * 0.04,
    }
    out = kernel(**ins)
    print("ran ok", out.shape, out.dtype)


# revision 33
# speedup vs baseline: 1.0302x; 1.0302x over previous
"""Depth-masked 3-branch 3x3 conv (Conv2.5D) on 8 TRN2 NeuronCores.

Data-parallel over N=8 images (1 image/core). Per core, v2 design:
  - x is host-prepped into two bf16 slabs tA=[x; x+1], tB=[x; x+128]
    (zero-padded), DMA'd straight to SBUF: no on-chip staging/conversion
  - phi in {0,1,2,3} encodes the active branch per (tap,pixel); computed
    once in f32 on DVE, collapsed to 8 pixel-major rc rows (one per tap)
  - per-chunk phi broadcast (row -> 64 partitions, both taps of a pair)
    is a single SBUF->SBUF DMA with a 0-stride middle dim: no PE/Act work
  - pair bases: p0,p1 E-basis {phi*x, E2*x, E3*x} (E_j built by DVE
    tensor_scalar at 4x rate); p2,p3 sign-basis {phi*x, s2*x, s3*x, x}
    (signs from Act, raw-x group free) -> 15 GEMM groups, 60 matmuls
    per 2048-px superchunk
  - multiply split: phi*x of p1,p2,p3 on Pool; the other 9 on DVE
  - out accumulated in PSUM, Act-copied to bf16, host casts to f32
"""

import sys

sys.path.insert(0, "/opt/trn_rl_repo")

import numpy as np
import ml_dtypes

import concourse.bass as bass
import concourse.mybir as mybir
from concourse.bass_utils import run_bass_kernel_spmd
from concourse import tile
from concourse.vector_clock import VectorClock, ScopedClock

F32 = mybir.dt.float32
BF16 = mybir.dt.bfloat16
AF = mybir.ActivationFunctionType
ALU = mybir.AluOpType

N_IMG, C, O, H, W = 8, 64, 64, 128, 128
L = H * W
CHUNK = 2048
NCHUNK = L // CHUNK
BASE = 144  # pad on each side of the x slabs (window reads span +-129)
XW = BASE + L + BASE
# tap k = 3*(dh+1)+(dw+1); flat pixel offset dh*W+dw
OFF = [(k // 3 - 1) * W + (k % 3 - 1) for k in range(9)]
# tap pairs (ka,kb): off(kb)-off(ka) == 1 -> tA, == 128 -> tB
PAIRS = [(0, 1), (7, 8), (2, 5), (3, 6)]
PAIR_SRC = ["A", "A", "B", "B"]
NGRP = 13  # center + 3 per pair


def _patched_drain_and_barrier(self, tick_clock, wait_clock):
    # stock version puts every live sem wait on one drain -> walrus
    # "Too many sync wait commands"; emit one single-wait NOP per sem.
    ticks = list(tick_clock.global_clock)
    n = len(ticks)
    for i, t in enumerate(ticks):
        if t > 0:
            vec = [0] * n
            vec[i] = t
            nop = self.nc.sync.nop()
            wait_clock.add_sem_waits(nop.ins, ScopedClock({None: VectorClock(vec)}))
    self.nc.sync.drain()
    self.nc.all_engine_barrier()
    popped = self.nc._tile_sem_poison_stack.pop()
    assert popped is self._sem_poison
    self.nc.clear_and_free_semaphores(list(self.sems.allocated().values()))
    self.nc.all_engine_barrier()


tile.TileContext._drain_and_barrier = _patched_drain_and_barrier


def _split_excess_waits(nc, noop_cls, max_waits=1):
    # this walrus build rejects >1 sync-wait on several instruction
    # structs; hoist extras onto same-engine NoOps placed just before.
    for fn in nc.m.functions:
        for blk in fn.blocks:
            idx = 0
            while idx < len(blk.instructions):
                inst = blk.instructions[idx]
                si = inst.sync_info
                if si is not None and len(si.on_wait) > max_waits:
                    waits = list(si.on_wait)
                    si.on_wait = waits[-max_waits:]
                    pos = idx
                    for w in waits[:-max_waits]:
                        nop = noop_cls(
                            name=nc.get_next_instruction_name(), ins=[], outs=[]
                        )
                        nop.engine = inst.engine
                        nop.sync_info = mybir.SyncInfo(on_wait=[w], on_update=[])
                        nc.register_instruction(nop)
                        blk.instructions.insert(pos, nop)
                        pos += 1
                        idx += 1
                idx += 1


def _build_graph():
    nc = bass.Bass()
    xt_d = nc.declare_dram_parameter("xt", [128, 2 * XW], BF16, isOutput=False)
    # host-packed: cols 0:390 = dsh (3 shifted depth copies, zero-edged),
    # col 390 = 1/fx per partition
    dsh_d = nc.declare_dram_parameter("dshp", [128, 391], F32, isOutput=False)
    wp_d = nc.declare_dram_parameter("wp", [128, NGRP * 64], BF16, isOutput=False)
    out_d = nc.declare_dram_parameter("out", [O, L], BF16, isOutput=True)

    with tile.TileContext(nc) as tc:
        WIN = CHUNK + 264  # slab window: [BASE+c0-132, BASE+c0+CHUNK+132)
        with (
            tc.tile_pool(name="big", bufs=1) as big,
            tc.tile_pool(name="slab", bufs=3) as slb,
            tc.tile_pool(name="mask", bufs=1) as mk,
            tc.tile_pool(name="scr", bufs=3) as scr,
            tc.tile_pool(name="rrep", bufs=10) as rrp,
            tc.tile_pool(name="xm", bufs=8) as xmp,
            tc.tile_pool(name="sgn", bufs=6) as sgp,
            tc.tile_pool(name="ebp", bufs=3) as ebp,
            tc.tile_pool(name="outp", bufs=2) as outp,
            tc.tile_pool(name="psum", bufs=2, space=bass.MemorySpace.PSUM) as psp,
        ):
            # ---- x slabs: host-prepped bf16, zero-padded; tA rows 0:64 =
            # x, 64:128 = x shifted +1; tB rows 64:128 = x shifted +128.
            # Per-superchunk windows in a ring: tile cols [0,WIN) = tA
            # window, [WIN,2*WIN) = tB window; col 132 = global BASE+c0 ----
            def load_slabs(ci):
                g0 = BASE + ci * CHUNK - 132
                xw = slb.tile([128, 2 * WIN], BF16, tag="xw")
                src = bass.AP(
                    xt_d[:].tensor, xt_d[:].offset + g0,
                    [list(xt_d[:].ap[0]), [XW, 2], [1, WIN]],
                )
                dst = bass.AP(
                    xw[:].tensor, xw[:].offset,
                    [list(xw[:].ap[0]), [WIN, 2], [1, WIN]],
                )
                nc.sync.dma_start(dst, src)
                return xw

            # depth DMA first: it heads the critical path (phi encode)
            dshp = mk.tile([128, 391], F32)
            nc.sync.dma_start(dshp[:], dsh_d[:])
            rfx = dshp[:, 390:391]
            wp = big.tile([128, NGRP * 64], BF16)
            nc.scalar.dma_start(wp[:], wp_d[:])
            slab_tiles = {0: load_slabs(0), 1: load_slabs(1)}
            # per-partition bias vectors for the AF.Relu basis planes:
            # relu(2*phi-3) = {0,0,1,3}, relu(2*phi-5) = {0,0,0,1}
            b3 = mk.tile([128, 1], F32)
            nc.vector.memset(b3[:], -3.0)
            b5 = mk.tile([128, 1], F32)
            nc.vector.memset(b5[:], -5.0)

            # ---- depth -> phi encoding, split into two 4-tap halves so
            # pair p0/p1 broadcasts start while p2/p3 still encode.
            # Half A = taps (0,1,7,8) = pairs p0,p1: dsh offsets
            # {0,1,261,262}; half B = taps (2,5,3,6) = pairs p2,p3:
            # offsets {2,132,130,260} ----
            g = mk.tile([128, 128], F32)
            nc.vector.tensor_scalar(g[:], dshp[:, 131:259], rfx, None, ALU.mult)
            rg = mk.tile([128, 128], F32)
            nc.vector.reciprocal(rg[:], g[:])

            def _win(base, offset, dims):
                return bass.AP(
                    base.tensor, offset, [list(base.ap[0])] + [list(d) for d in dims]
                )

            rc = big.tile([8, L], BF16)
            rgb4 = _win(rg[:], 0, [(0, 4), (1, 128)])
            cent4 = _win(dshp[:], 131, [(0, 4), (1, 128)])

            def encode_half(dims, off):
                dcol = _win(dshp[:], off, dims + [(1, 128)])
                et = scr.tile([128, 512], F32, tag="u")
                nc.vector.tensor_tensor(et[:], dcol, cent4, ALU.subtract)
                tq = scr.tile([128, 512], F32, tag="t")
                nc.vector.tensor_tensor(tq[:], et[:], rgb4, ALU.mult)
                ua = scr.tile([128, 512], F32, tag="u")
                nc.vector.tensor_scalar(ua[:], tq[:], -1.5, None, ALU.is_ge)
                ub = scr.tile([128, 512], F32, tag="u")
                nc.vector.scalar_tensor_tensor(
                    ub[:], tq[:], -0.5, ua[:], ALU.is_ge, ALU.add
                )
                uc = scr.tile([128, 512], F32, tag="u")
                nc.vector.scalar_tensor_tensor(
                    uc[:], tq[:], 0.5, ub[:], ALU.is_ge, ALU.add
                )
                renc = scr.tile([128, 512], BF16, tag="r")
                nc.vector.scalar_tensor_tensor(
                    renc[:], tq[:], 1.5, uc[:], ALU.is_lt, ALU.mult
                )
                return renc

            def collapse_pair(renc, i0, r0):
                for i in (i0, i0 + 1):
                    r = r0 + (i - i0)
                    eng = nc.sync if i % 2 == 0 else nc.scalar
                    eng.dma_start(
                        rc[r : r + 1, :], renc[:, i * 128 : (i + 1) * 128]
                    )

            # phi broadcast for superchunk ci, pairs [plo,phi): one DMA per
            # pair; row 2p -> partitions 0:64, row 2p+1 -> 64:128
            def bcast(ci, plo=0, phi=4, rrs=None):
                c0 = ci * CHUNK
                rrs = rrs if rrs is not None else [None] * 4
                for p in range(plo, phi):
                    rr = rrp.tile([128, CHUNK], BF16, tag="rr")
                    b = rc[2 * p : 2 * p + 2, c0 : c0 + CHUNK]
                    src = bass.AP(
                        b.tensor, b.offset,
                        [list(b.ap[0]), [0, 64], [1, CHUNK]],
                    )
                    eng = nc.sync if p % 2 == 0 else nc.scalar
                    eng.dma_start(rr[:], src)
                    rrs[p] = rr
                return rrs

            ra = encode_half([(261, 2), (1, 2)], 0)
            collapse_pair(ra, 0, 0)
            pipe = bcast(0, 0, 1)
            collapse_pair(ra, 2, 2)
            bcast(0, 1, 2, pipe)
            rb = encode_half([(128, 2), (130, 2)], 2)
            collapse_pair(rb, 0, 4)
            bcast(0, 2, 3, pipe)
            collapse_pair(rb, 2, 6)
            bcast(0, 3, 4, pipe)

            # logical groups: 0=center; pair p slots 1+3p..3+3p = {f1,f2,f3}
            # f1 = phi*x; p0,p1: f2,f3 = E2*x, E3*x (DVE tensor_scalar);
            # p2,p3: f2,f3 = relu(2phi-3)*x, relu(2phi-5)*x (Act planes)
            pending = None
            for ci in range(NCHUNK):
                c0 = ci * CHUNK
                rr = pipe
                xw = slab_tiles.pop(ci)

                def xwin_of(p):
                    off = (0 if PAIR_SRC[p] == "A" else WIN) + 132
                    off += OFF[PAIRS[p][0]]
                    return xw[:, off : off + CHUNK]

                xms = {}

                # ---- Pool: phi*x for p1, p2, p3 (slow; consumed last) ----
                for p in (1, 2, 3):
                    xm = xmp.tile([128, CHUNK], BF16, tag="xm")
                    nc.gpsimd.tensor_tensor(xm[:], rr[p][:], xwin_of(p), ALU.mult)
                    xms[1 + 3 * p] = xm

                # ---- Act: relu basis planes: (p,j) in (1,2),(2,2),(2,3),
                # (3,2),(3,3); p1 f3 stays an E3 plane on DVE ----
                sg = {}
                for p, j, bv in (
                    (1, 2, b3), (2, 2, b3), (2, 3, b5), (3, 2, b3), (3, 3, b5)
                ):
                    s = sgp.tile([128, CHUNK], BF16, tag="sg")
                    nc.scalar.activation(
                        s[:], rr[p][:], AF.Relu, bias=bv[:], scale=2.0
                    )
                    sg[(p, j)] = s

                # previous super's PSUM -> SBUF copy + store, emitted after
                # this super's Act planes so it never delays them
                if pending is not None:
                    pacc, pc0 = pending
                    osb = outp.tile([O, CHUNK], BF16, tag="osb")
                    nc.scalar.activation(osb[:], pacc[:], AF.Copy)
                    nc.sync.dma_start(out_d[:, pc0 : pc0 + CHUNK], osb[:])

                # ---- DVE: E planes + 9 multiplies ----
                def dve_tt(gid, a, p):
                    xm = xmp.tile([128, CHUNK], BF16, tag="xm")
                    nc.vector.tensor_tensor(xm[:], a[:], xwin_of(p), ALU.mult)
                    xms[gid] = xm

                # E planes first: DVE then streams multiplies gap-free at
                # a rate above PE consumption, so PE keeps a backlog
                ebs = {}
                for p, j in ((0, 2), (0, 3), (1, 3)):
                    eb = ebp.tile([128, CHUNK], BF16, tag="eb")
                    nc.vector.tensor_scalar(
                        eb[:], rr[p][:], j - 0.5, None, ALU.is_ge
                    )
                    ebs[(p, j)] = eb
                dve_tt(1, rr[0], 0)  # p0 f1 = phi*x
                dve_tt(2, ebs[(0, 2)], 0)
                dve_tt(3, ebs[(0, 3)], 0)
                dve_tt(6, ebs[(1, 3)], 1)
                dve_tt(5, sg[(1, 2)], 1)
                dve_tt(8, sg[(2, 2)], 2)
                if ci + 1 < NCHUNK:
                    pipe = bcast(ci + 1)
                if ci + 2 < NCHUNK:
                    slab_tiles[ci + 2] = load_slabs(ci + 2)
                dve_tt(9, sg[(2, 3)], 2)
                dve_tt(11, sg[(3, 2)], 3)
                dve_tt(12, sg[(3, 3)], 3)

                # ---- matmuls: 13 groups x 4 PSUM banks ----
                # order: center first (free rhs), DVE tiles in build order,
                # Pool tiles last
                acc = psp.tile([O, CHUNK], F32)
                mm_rhs = [(0, xw[:, 132 : 132 + CHUNK])]
                for gid in (1, 2, 3, 6, 5, 8, 9, 11, 12):
                    mm_rhs.append((gid, xms[gid][:]))
                for gid in (4, 7, 10):  # Pool phi*x tiles
                    mm_rhs.append((gid, xms[gid][:]))

                nmm = len(mm_rhs)
                MMN = 512
                for oi, (gid, rhs) in enumerate(mm_rhs):
                    for h in range(CHUNK // MMN):
                        nc.tensor.matmul(
                            acc[:, h * MMN : (h + 1) * MMN],
                            wp[:, gid * 64 : (gid + 1) * 64],
                            bass.AP(
                                rhs.tensor,
                                rhs.offset + h * MMN,
                                [list(rhs.ap[0])] + [[1, MMN]],
                            ),
                            start=(oi == 0),
                            stop=(oi == nmm - 1),
                        )
                pending = (acc, c0)

            pacc, pc0 = pending
            osb = outp.tile([O, CHUNK], BF16, tag="osb")
            nc.scalar.activation(osb[:], pacc[:], AF.Copy)
            nc.sync.dma_start(out_d[:, pc0 : pc0 + CHUNK], osb[:])

    noop_cls = type(nc.sync.nop().ins)
    _split_excess_waits(nc, noop_cls, max_waits=1)
    return nc


def _bf(a):
    return a.astype(ml_dtypes.bfloat16).astype(np.float32)


def _prep_weights(w0, w1, w2):
    # basis-transformed weights; see module docstring.
    # E-basis (p0,p1):  V1=W2, V2=W1-2W2, V3=W0-W1-W2 for {phi,E2,E3}
    # sign (p2,p3) {f1,s2,s3,xr}: U1=W2, U2=.5W1-W2, U3=.5(W0-W1-W2),
    #   U4=U2+U3 (from the bf16-rounded terms so phi=0 pixels cancel
    #   exactly in the f32 PSUM accumulation)
    ws = [w0.reshape(O, C, 9), w1.reshape(O, C, 9), w2.reshape(O, C, 9)]
    W0, W1, W2 = ws
    wp = np.zeros((128, NGRP * 64), dtype=np.float32)

    def put(gi, ka, kb, arr):
        wp[0:64, gi * 64 : (gi + 1) * 64] = arr[:, :, ka].T
        if kb is not None:
            wp[64:128, gi * 64 : (gi + 1) * 64] = arr[:, :, kb].T

    put(0, 4, None, W1)  # center
    # E-basis (p0,p1) over {phi, E2, E3}; relu-basis (p2,p3) over
    # {phi, relu(2phi-3), relu(2phi-5)} = {(1,2,3),(0,1,3),(0,0,1)} at
    # phi=1,2,3 -> R = [W2, W1-2W2, W0-3W1+3W2]
    V = [W2, W1 - 2 * W2, W0 - W1 - W2]
    R = [W2, W1 - 2 * W2, W0 - 3 * W1 + 3 * W2]
    for p in range(4):
        ka, kb = PAIRS[p]
        basis = V if p == 0 else R
        for j in range(3):
            put(1 + p * 3 + j, ka, kb, basis[j])
    return wp.astype(ml_dtypes.bfloat16)


def _prep_dsh(dep, fxi):
    # [128, 391] f32: 3 x 130-col blocks of row-shifted depth (dh=-1,0,+1,
    # zero edges, 1-col left pad within each block) + col 390 = 1/fx
    d = np.zeros((128, 391), dtype=np.float32)
    d[1:128, 1:129] = dep[0:127]
    d[:, 131:259] = dep
    d[0:127, 261:389] = dep[1:128]
    d[:, 390] = 1.0 / fxi
    return d


def _prep_x(x):
    # x: [C, L] f32 -> [128, 2*XW] bf16: tA = [x; x+1], tB = [x; x+128]
    xp = np.zeros((C, XW + 130), dtype=np.float32)
    xp[:, BASE : BASE + L] = x
    xt = np.zeros((128, 2 * XW), dtype=np.float32)
    xt[0:64, 0:XW] = xp[:, 0:XW]
    xt[64:128, 0:XW] = xp[:, 1 : XW + 1]
    xt[0:64, XW:] = xp[:, 0:XW]
    xt[64:128, XW:] = xp[:, 128 : XW + 128]
    return xt.astype(ml_dtypes.bfloat16)


def kernel(x, depth, fx, weight_0, weight_1, weight_2, _trace=False):
    x = np.asarray(x, dtype=np.float32)
    depth = np.asarray(depth, dtype=np.float32)
    fx = np.asarray(fx, dtype=np.float32)
    wp = _prep_weights(
        np.asarray(weight_0, np.float32),
        np.asarray(weight_1, np.float32),
        np.asarray(weight_2, np.float32),
    )
    in_maps = []
    for i in range(N_IMG):
        in_maps.append(
            {
                "xt": _prep_x(x[i].reshape(C, L)),
                "dshp": _prep_dsh(depth[i, 0], fx[i]),
                "wp": wp,
            }
        )
    nc = _build_graph()
    res = run_bass_kernel_spmd(nc, in_maps, core_ids=list(range(N_IMG)), trace=_trace)
    out = np.stack(
        [
            res.results[i]["out"].astype(np.float32).reshape(O, H, W)
            for i in range(N_IMG)
        ]
    )
    if _trace:
        return out, res
    return out


if __name__ == "__main__":
    rng = np.random.default_rng(0)
    ins = {
        "x": rng.standard_normal((N_IMG, C, H, W), dtype=np.float32),
        "depth": (1.0 + 9.0 * rng.random((N_IMG, 1, H, W))).astype(np.float32),
        "fx": (400.0 + 200.0 * rng.random(N_IMG)).astype(np.float32),
        "weight_0": rng.standard_normal((O, C, 3, 3), dtype=np.float32) * 0.04,
        "weight_1": rng.standard_normal((O, C, 3, 3), dtype=np.float32) * 0.04,
        "weight_2": rng.standard_normal((O, C, 3, 3), dtype=np.float32) * 0.04,
    }
    out = kernel(**ins)
    print("ran ok", out.shape, out.dtype)
